# revision 11
# baseline (speedup 1.0000x reference)
"""Trainium2 Bass kernel for nn_ChannelProcessing.

Reference computation (per batch element b, x [N=4096, C=512], nh=8, d=64):
  q   = x @ Wq.T                          # [N, C]
  q_sm = softmax(q, axis=N) (per column)  # [N, C]
  k_sm = softmax(x, axis=N) (per column)  # [N, C]
  kp[n, h] = mean_{dd} k_sm[n, h*64+dd]   # [N, nh]
  S[c] = sum_n q_sm[n, c] * kp[n, h(c)]   # [C]
  attn[c] = sigmoid(S[c]) * temp[h(c)]    # [C]
  m  = LayerNorm(gelu(x @ W1.T + b1) @ W2.T + b2) * g + b   # [N, C]
  out[n, c]    = m[n, c] * attn[c]        # [N, C]
  attn_v[c, n] = out[n, c]                # [C, N]  (=[nh, d, N])

Sharding: data-parallel over batch B=8 across the 8 NeuronCores.
kernel(**inputs) takes the FULL inputs and returns (out, attn_v) like the
reference.

On-chip layout strategy per core:
  - x is transposed once on the PE (fp32r transpose mode) to xT [C, N] so the
    two softmaxes reduce along the free axis and the linears chain as
    transposed-domain matmuls with fp32r (1 cycle/row) throughput.
  - The attention scalar S is a fused multiply+reduce (tensor_tensor_reduce)
    of exp(qT) against a head-pooled broadcast of k_sm built with one PE
    matmul per chunk (block-diagonal mask x 1/(64*denom_k)).
  - The MLP output m is computed in natural layout [N, C] (h1T tiles act as
    the stationary operand) so LayerNorm stats use bn_stats/bn_aggr and the
    affine is a fused dual-scalar tensor_scalar during PSUM eviction.
  - out = m0 * broadcast(ln_g*attn) on GpSimd; attn_v tiles are PE transposes
    of m0 scaled per-partition during eviction.
"""

import os
import sys

import numpy as np

for _p in ("/opt/trn_rl_repo",):
    if os.path.isdir(_p) and _p not in sys.path:
        sys.path.insert(0, _p)

import concourse.bass as bass  # noqa: E402,F401
import concourse.mybir as mybir  # noqa: E402
import concourse.tile as tile  # noqa: E402
from concourse import bacc  # noqa: E402
from concourse.bass import ts  # noqa: E402
from concourse.bass_utils import run_bass_kernel_spmd  # noqa: E402
from concourse.masks import make_identity  # noqa: E402

F32 = mybir.dt.float32
F32R = mybir.dt.float32r
BF16 = mybir.dt.bfloat16
AF = mybir.ActivationFunctionType
ALU = mybir.AluOpType

N_CORES = 8
NH = 8
LN_EPS = 1e-5


def _r(ap):
    """View an fp32 AP as float32r for full-rate PE matmuls."""
    return ap.bitcast(F32R)


def build_program(n_tok: int, c_dim: int, flags: dict):
    """Emit the per-core Bass program and compile it."""
    max_phase = int(os.environ.get("K_PHASES", "9"))
    P = 128
    CT = c_dim // P           # c tiles (4)
    NT = n_tok // P           # token tiles (32)
    NCH = n_tok // 512        # 512-wide token chunks (8)
    GRP = NT // 4             # groups of 4 token tiles (8)

    has_fc1b = flags["has_fc1b"]
    has_fc2b = flags["has_fc2b"]
    has_lnb = flags["has_lnb"]

    nc = bacc.Bacc(
        "TRN2", target_bir_lowering=False, debug=False, num_devices=N_CORES
    )

    x_d = nc.dram_tensor("x", [n_tok, c_dim], F32, kind="ExternalInput").ap()
    wq_d = nc.dram_tensor("wq_t", [c_dim, c_dim], F32R, kind="ExternalInput").ap()
    w1_d = nc.dram_tensor("w1_t", [c_dim, c_dim], F32R, kind="ExternalInput").ap()
    w2_d = nc.dram_tensor("w2_t", [c_dim, c_dim], F32R, kind="ExternalInput").ap()
    aux_d = nc.dram_tensor("aux", [5, c_dim], F32, kind="ExternalInput").ap()
    out1_d = nc.dram_tensor("out1", [n_tok, c_dim], F32, kind="ExternalOutput").ap()
    out2_d = nc.dram_tensor("out2", [c_dim, n_tok], F32, kind="ExternalOutput").ap()

    with tile.TileContext(nc) as tc:
      try:
        with (
            tc.tile_pool(name="persist", bufs=1) as persist,
            tc.tile_pool(name="stats", bufs=16) as stats_pool,
        ):
            # ---------------- setup ----------------
            ident = persist.tile([P, P], F32)
            make_identity(nc, ident[:])
            mask64 = persist.tile([P, P], F32)
            nc.gpsimd.memset(mask64[:], 0.0)
            nc.gpsimd.memset(mask64[0:64, 0:64], 1.0)
            nc.gpsimd.memset(mask64[64:128, 64:128], 1.0)
            ones_row = persist.tile([1, P], F32)
            nc.vector.memset(ones_row[:], 1.0)
            eps_col = persist.tile([P, 1], F32)
            nc.vector.memset(eps_col[:], LN_EPS)

            wq_sb, w1_sb, w2_sb = [], [], []
            for kt in range(CT):
                for nm, lst, src in (
                    ("wq", wq_sb, wq_d),
                    ("w1", w1_sb, w1_d),
                    ("w2", w2_sb, w2_d),
                ):
                    t = persist.tile([P, c_dim], F32R, name=f"{nm}sb{kt}")
                    nc.sync.dma_start(out=t[:], in_=src[ts(kt, P), :])
                    lst.append(t)

            # aux rows: 0=fc1_b 1=fc2_b 2=ln_g 3=ln_b 4=temp_col
            aux_cols = persist.tile([P, 5 * CT], F32)

            def aux_col(row, ct_):
                return aux_cols[:, row * CT + ct_ : row * CT + ct_ + 1]

            for row in range(5):
                for ct_ in range(CT):
                    src = aux_d[row : row + 1, ts(ct_, P)].rearrange(
                        "o (p f) -> (o p) f", p=P
                    )
                    nc.sync.dma_start(out=aux_col(row, ct_), in_=src)
            fc2b_row = persist.tile([1, c_dim], F32)
            if has_fc2b:
                nc.sync.dma_start(out=fc2b_row[:], in_=aux_d[1:2, :])

            attn4 = persist.tile([P, CT], F32)
            ga4 = persist.tile([P, CT], F32)
            ba4 = persist.tile([P, CT], F32)
            ga_bcast = persist.tile([P, c_dim], F32)
            ba_bcast = persist.tile([P, c_dim], F32)

            with tc.tile_pool(name="xTp", bufs=1) as xT_pool:
                xT = [
                    xT_pool.tile([P, n_tok], F32R, name=f"xT{i}") for i in range(CT)
                ]

                # ---------------- P1: x -> xT (PE transpose) ----------------
                if max_phase < 1:
                    raise _PhaseStop
                with (
                    tc.tile_pool(name="xnat", bufs=6) as xnat_pool,
                    tc.tile_pool(name="psT", bufs=2, space="PSUM") as psT,
                ):
                    for g in range(GRP):
                        xn = []
                        for j in range(4):
                            t = xnat_pool.tile([P, c_dim], F32, tag="xn")
                            nc.sync.dma_start(out=t[:], in_=x_d[ts(4 * g + j, P), :])
                            xn.append(t)
                        for ct_ in range(CT):
                            ps = psT.tile([P, 512], F32, tag="psT")
                            for j in range(4):
                                nc.tensor.transpose(
                                    ps[:, ts(j, P)],
                                    xn[j][:, ts(ct_, P)],
                                    ident[:],
                                )
                            nc.any.tensor_copy(xT[ct_][:, ts(g, 512)], ps[:])

                # ---------------- P2+P3: k/q branches -> attn ----------------
                if max_phase < 2:
                    raise _PhaseStop
                with (
                    tc.tile_pool(name="kq", bufs=1) as kq_pool,
                    tc.tile_pool(name="expq", bufs=2) as expq_pool,
                    tc.tile_pool(name="psq", bufs=2, space="PSUM") as psq_pool,
                    tc.tile_pool(name="pskp", bufs=2, space="PSUM") as pskp_pool,
                ):
                    expk = [
                        kq_pool.tile([P, n_tok], BF16, name=f"expk{i}")
                        for i in range(CT)
                    ]
                    denomk4 = stats_pool.tile([P, CT], F32, tag="den")
                    for ct_ in range(CT):
                        nc.scalar.activation(
                            out=expk[ct_][:],
                            in_=xT[ct_][:],
                            func=AF.Exp,
                            accum_out=denomk4[:, ct_ : ct_ + 1],
                        )
                    rk4 = stats_pool.tile([P, CT], F32, tag="den")
                    nc.vector.reciprocal(out=rk4[:], in_=denomk4[:])
                    dfull = []
                    for ct_ in range(CT):
                        dd = kq_pool.tile([P, P], BF16, name=f"dfull{ct_}")
                        nc.vector.tensor_scalar(
                            out=dd[:],
                            in0=mask64[:],
                            scalar1=rk4[:, ct_ : ct_ + 1],
                            scalar2=1.0 / 64.0,
                            op0=ALU.mult,
                            op1=ALU.mult,
                        )
                        dfull.append(dd)

                    if max_phase < 3:
                        raise _PhaseStop
                    denq4 = stats_pool.tile([P, CT], F32, tag="den")
                    sraw4 = stats_pool.tile([P, CT], F32, tag="den")
                    ttr_scr = kq_pool.tile([P, 512], F32, name="ttr_scr")
                    for ct_ in range(CT):
                        expq = expq_pool.tile([P, n_tok], F32, tag="expq")
                        sums8 = stats_pool.tile([P, NCH], F32, tag="sums")
                        for nch in range(NCH):
                            psq = psq_pool.tile([P, 512], F32, tag="psq")
                            for kt in range(CT):
                                nc.tensor.matmul(
                                    psq[:],
                                    wq_sb[kt][:, ts(ct_, P)],
                                    xT[kt][:, ts(nch, 512)],
                                    start=(kt == 0),
                                    stop=(kt == CT - 1),
                                )
                            nc.scalar.activation(
                                out=expq[:, ts(nch, 512)],
                                in_=psq[:],
                                func=AF.Exp,
                                accum_out=sums8[:, nch : nch + 1],
                            )
                        nc.vector.tensor_reduce(
                            out=denq4[:, ct_ : ct_ + 1],
                            in_=sums8[:],
                            axis=mybir.AxisListType.X,
                            op=ALU.add,
                        )
                        if max_phase < 4:
                            continue
                        p4mode = os.environ.get("K_P4MODE", "full")
                        sp8 = None
                        if p4mode != "mm":
                            sp8 = stats_pool.tile([P, NCH], F32, tag="sums")
                        for nch in range(NCH):
                            pskp = pskp_pool.tile([P, 512], F32, tag="pskp")
                            nc.tensor.matmul(
                                pskp[:],
                                dfull[ct_][:],
                                expk[ct_][:, ts(nch, 512)],
                                start=True,
                                stop=True,
                            )
                            if p4mode == "mm":
                                continue
                            prod = expq_pool.tile([P, 512], F32, tag="prod")
                            nc.vector.tensor_mul(
                                out=prod[:],
                                in0=expq[:, ts(nch, 512)],
                                in1=pskp[:],
                            )
                            nc.scalar.activation(
                                out=ttr_scr[:],
                                in_=prod[:],
                                func=AF.Copy,
                                accum_out=sp8[:, nch : nch + 1],
                            )
                        if sp8 is not None:
                            nc.vector.tensor_reduce(
                                out=sraw4[:, ct_ : ct_ + 1],
                                in_=sp8[:],
                                axis=mybir.AxisListType.X,
                                op=ALU.add,
                            )

                    # attn = sigmoid(S/denq) * temp ; ga/ba = attn * ln_g/ln_b
                    if max_phase < 5:
                        raise _PhaseStop
                    rq4 = stats_pool.tile([P, CT], F32, tag="den")
                    nc.vector.reciprocal(out=rq4[:], in_=denq4[:])
                    s4 = stats_pool.tile([P, CT], F32, tag="den")
                    nc.vector.tensor_mul(out=s4[:], in0=sraw4[:], in1=rq4[:])
                    nc.scalar.activation(out=attn4[:], in_=s4[:], func=AF.Sigmoid)
                    if flags["has_temp"]:
                        tmp4 = stats_pool.tile([P, CT], F32, tag="den")
                        for ct_ in range(CT):
                            nc.vector.tensor_copy(
                                out=tmp4[:, ct_ : ct_ + 1], in_=aux_col(4, ct_)
                            )
                        nc.vector.tensor_mul(out=attn4[:], in0=attn4[:], in1=tmp4[:])
                    if flags["has_lng"]:
                        g4 = stats_pool.tile([P, CT], F32, tag="den")
                        for ct_ in range(CT):
                            nc.vector.tensor_copy(
                                out=g4[:, ct_ : ct_ + 1], in_=aux_col(2, ct_)
                            )
                        nc.vector.tensor_mul(out=ga4[:], in0=attn4[:], in1=g4[:])
                    else:
                        nc.vector.tensor_copy(out=ga4[:], in_=attn4[:])
                    if has_lnb:
                        b4 = stats_pool.tile([P, CT], F32, tag="den")
                        for ct_ in range(CT):
                            nc.vector.tensor_copy(
                                out=b4[:, ct_ : ct_ + 1], in_=aux_col(3, ct_)
                            )
                        nc.vector.tensor_mul(out=ba4[:], in0=attn4[:], in1=b4[:])

                    # ga_bcast [128, C]: ga replicated along partitions
                    def bcast_cols(col4, dst, tag):
                        psg = psq_pool.tile([P, 512], F32, tag="psq")
                        for ct_ in range(CT):
                            nc.tensor.transpose(
                                psg[0:1, ts(ct_, P)], col4[:, ct_ : ct_ + 1], ident[:]
                            )
                        row = kq_pool.tile([1, c_dim], F32, name=f"row_{tag}")
                        nc.any.tensor_copy(row[0:1, :], psg[0:1, :c_dim])
                        psb = pskp_pool.tile([P, 512], F32, tag="pskp")
                        nc.tensor.matmul(
                            psb[:, :c_dim],
                            ones_row[:],
                            row[:],
                            start=True,
                            stop=True,
                        )
                        nc.any.tensor_copy(dst[:], psb[:, :c_dim])

                    if max_phase < 6:
                        raise _PhaseStop
                    bcast_cols(ga4, ga_bcast, "ga")
                    if has_lnb:
                        bcast_cols(ba4, ba_bcast, "ba")

                # ---------------- P4: h1T = gelu(W1 @ xT + b1) ----------------
                if max_phase < 7:
                    raise _PhaseStop
                with (
                    tc.tile_pool(name="h1", bufs=1) as h1_pool,
                    tc.tile_pool(name="psh", bufs=2, space="PSUM") as psh_pool,
                ):
                    h1T = [
                        h1_pool.tile([P, n_tok], F32R, name=f"h1T{i}")
                        for i in range(CT)
                    ]
                    for jt in range(CT):
                        for nch in range(NCH):
                            psh = psh_pool.tile([P, 512], F32, tag="psh")
                            for kt in range(CT):
                                nc.tensor.matmul(
                                    psh[:],
                                    w1_sb[kt][:, ts(jt, P)],
                                    xT[kt][:, ts(nch, 512)],
                                    start=(kt == 0),
                                    stop=(kt == CT - 1),
                                )
                            if has_fc1b:
                                nc.scalar.activation(
                                    out=h1T[jt][:, ts(nch, 512)],
                                    in_=psh[:],
                                    func=AF.Gelu,
                                    bias=aux_col(0, jt),
                                )
                            else:
                                nc.scalar.activation(
                                    out=h1T[jt][:, ts(nch, 512)],
                                    in_=psh[:],
                                    func=AF.Gelu,
                                )

                    # ------------ P5: m, layernorm, outputs ------------
                    if max_phase < 8:
                        raise _PhaseStop
                    with (
                        tc.tile_pool(name="m0p", bufs=6) as m0_pool,
                        tc.tile_pool(name="evac", bufs=3) as evac_pool,
                        tc.tile_pool(name="psm", bufs=2, space="PSUM") as psm_pool,
                        tc.tile_pool(name="psav", bufs=2, space="PSUM") as psav_pool,
                    ):
                        for g in range(GRP):
                            m0s = []
                            for j in range(4):
                                nt = 4 * g + j
                                psm = psm_pool.tile([P, 512], F32, tag="psm")
                                for jt in range(CT):
                                    nc.tensor.matmul(
                                        psm[:, :c_dim],
                                        h1T[jt][:, ts(nt, P)],
                                        w2_sb[jt][:],
                                        start=(jt == 0),
                                        stop=(jt == CT - 1 and not has_fc2b),
                                    )
                                if has_fc2b:
                                    nc.tensor.matmul(
                                        psm[:, :c_dim],
                                        ones_row[:],
                                        fc2b_row[:],
                                        start=False,
                                        stop=True,
                                    )
                                st6 = stats_pool.tile([P, 6], F32, tag="st6")
                                nc.vector.bn_stats(out=st6[:], in_=psm[:, :c_dim])
                                mv = stats_pool.tile([P, 2], F32, tag="mv")
                                nc.vector.bn_aggr(out=mv[:], in_=st6[:])
                                rstd = stats_pool.tile([P, 1], F32, tag="rstd")
                                nc.scalar.activation(
                                    out=rstd[:],
                                    in_=mv[:, 1:2],
                                    func=AF.Sqrt,
                                    bias=eps_col[:],
                                )
                                nc.vector.reciprocal(out=rstd[:], in_=rstd[:])
                                m0 = m0_pool.tile([P, c_dim], F32, tag="m0")
                                nc.vector.tensor_scalar(
                                    out=m0[:],
                                    in0=psm[:, :c_dim],
                                    scalar1=mv[:, 0:1],
                                    scalar2=rstd[:],
                                    op0=ALU.subtract,
                                    op1=ALU.mult,
                                )
                                m0s.append(m0)
                                o = evac_pool.tile([P, c_dim], F32, tag="o")
                                nc.gpsimd.tensor_mul(
                                    out=o[:], in0=m0[:], in1=ga_bcast[:]
                                )
                                if has_lnb:
                                    nc.gpsimd.tensor_add(
                                        out=o[:], in0=o[:], in1=ba_bcast[:]
                                    )
                                nc.sync.dma_start(out=out1_d[ts(nt, P), :], in_=o[:])
                            for ct_ in range(CT):
                                pst = psav_pool.tile([P, 512], F32, tag="psav")
                                for j in range(4):
                                    nc.tensor.transpose(
                                        pst[:, ts(j, P)],
                                        m0s[j][:, ts(ct_, P)],
                                        ident[:],
                                    )
                                av = evac_pool.tile([P, 512], F32, tag="av")
                                if has_lnb:
                                    nc.vector.tensor_scalar(
                                        out=av[:],
                                        in0=pst[:],
                                        scalar1=ga4[:, ct_ : ct_ + 1],
                                        scalar2=ba4[:, ct_ : ct_ + 1],
                                        op0=ALU.mult,
                                        op1=ALU.add,
                                    )
                                else:
                                    nc.scalar.activation(
                                        out=av[:],
                                        in_=pst[:],
                                        func=AF.Identity,
                                        scale=ga4[:, ct_ : ct_ + 1],
                                    )
                                nc.sync.dma_start(
                                    out=out2_d[ts(ct_, P), ts(g, 512)], in_=av[:]
                                )

      except _PhaseStop:
        pass
    nc.compile()
    return nc


class _PhaseStop(Exception):
    pass


_CACHE = {}


def _get_program(n_tok, c_dim, flags):
    key = (n_tok, c_dim, tuple(sorted(flags.items())))
    if key not in _CACHE:
        _CACHE[key] = build_program(n_tok, c_dim, flags)
    return _CACHE[key]


def _prep(x, Wq, temperature, fc1_w, fc1_b, fc2_w, fc2_b, ln_g, ln_b):
    """Host-side preprocessing shared by kernel() and the sim tests."""
    B, n_tok, c_dim = x.shape
    d = c_dim // NH
    x = np.ascontiguousarray(np.asarray(x, dtype=np.float32))
    wq_t = np.ascontiguousarray(np.asarray(Wq, np.float32).T)
    w1_t = np.ascontiguousarray(np.asarray(fc1_w, np.float32).T)
    w2_t = np.ascontiguousarray(np.asarray(fc2_w, np.float32).T)
    fc1_b = np.asarray(fc1_b, np.float32)
    fc2_b = np.asarray(fc2_b, np.float32)
    ln_g = np.asarray(ln_g, np.float32)
    ln_b = np.asarray(ln_b, np.float32)
    temp_col = np.repeat(np.asarray(temperature, np.float32).reshape(NH), d)
    aux = np.ascontiguousarray(
        np.stack([fc1_b, fc2_b, ln_g, ln_b, temp_col]).astype(np.float32)
    )
    flags = {
        "has_fc1b": bool(np.any(fc1_b != 0)),
        "has_fc2b": bool(np.any(fc2_b != 0)),
        "has_lng": bool(np.any(ln_g != 1)),
        "has_lnb": bool(np.any(ln_b != 0)),
        "has_temp": bool(np.any(temp_col != 1)),
    }
    in_maps = [
        {"x": x[b], "wq_t": wq_t, "w1_t": w1_t, "w2_t": w2_t, "aux": aux}
        for b in range(B)
    ]
    return in_maps, flags, (B, n_tok, c_dim, d)


def kernel(x, Wq, temperature, fc1_w, fc1_b, fc2_w, fc2_b, ln_g, ln_b, H, W):
    in_maps, flags, (B, n_tok, c_dim, d) = _prep(
        x, Wq, temperature, fc1_w, fc1_b, fc2_w, fc2_b, ln_g, ln_b
    )
    assert B == N_CORES, f"expected batch {N_CORES}, got {B}"
    nc = _get_program(n_tok, c_dim, flags)
    res = run_bass_kernel_spmd(nc, in_maps, list(range(N_CORES))).results
    out = np.stack([res[b]["out1"] for b in range(B)])
    attn_v = np.stack([res[b]["out2"] for b in range(B)]).reshape(B, NH, d, n_tok)
    return out, attn_v
